# revision 2
# baseline (speedup 1.0000x reference)
"""Deformable conv block (3x3 offset conv -> 3x3 deformable group conv), 8x trn2.

v3: 3x3 tent window (exact for |offset|<1; ~200 outlier (tap,pixel) events of
2M are patched on the host from the returned offsets), 96 q-rows instead of
240, q kept in SBUF (no DRAM staging), fold-tree reduction in fp16 (DVE 2x
mode; microbenchmarked at 0.52ns/elem vs 1.04 for tensor_reduce).

Sharding: data-parallel over (batch=2) x (H quarters=4) -> 8 cores; each core
gets a zero-padded slab (2-row/2-col halo) so sampling's zero-outside-image
semantics fall out of the padding.

Pipeline per core:
  Phase 1 (per row r): 3x3 offset conv on PE -> offsets [18, 320]; tent
    coefficients ty/tx = relu(1-|d-(u-1)|) via rep-matmul + 2 scalar
    activations each; q = ty*tx on DVE; pair-duplicating transpose via PE
    matmul with D[96,192] (D[j,2j]=D[j,2j+1]=1); drain to SBUF q2.
    Offsets are also DMAed out for the host-side outlier patch.
  Phase 2 (per col-tile, per row): T-slab build on PE (30 slots = 3ky x
    (9 (kx,v) pairs + pad), slot-major [slot,o] fp16 after scalar drain);
    5 DVE mults (one per row-offset a in {-2..2}) with pair-dup q reads;
    fp16 fold tree 96->3 then 2 final adds into f32; DMA out.
"""

import numpy as np
from contextlib import ExitStack

import concourse.bass as bass
import concourse.tile as tile
from concourse import bacc, mybir
from concourse import bass_utils
from concourse.bass import AP

# Problem constants
B, C, O, H, W = 2, 72, 72, 180, 320
NK = 9                # deform taps
OC = 18               # offset channels
PADC = 2
WP = W + 2 * PADC     # 324
NQ = 4
RS = H // NQ          # 45
HALO = 2
RSP = RS + 2 * HALO   # 49
NPIX_I = RSP * WP
FROWS = RS + 2        # feat slab rows (conv needs +-1)
NPIX_F = FROWS * WP
N_CORES = 8

F32 = mybir.dt.float32
F16 = mybir.dt.float16

# --- slot / coefficient layout -------------------------------------------
# Per tap-row ky, the 9 (kx, v) pairs sorted by column shift s = kx-1 + v-1.
SLOT9 = sorted(((kx - 1 + v - 1, kx, v) for kx in range(3) for v in range(3)))
NSPK = 10             # slots per ky: 9 real + 1 pad
NSLOT = 3 * NSPK      # 30
SPB = 6               # slots per PSUM bank (6*72=432 <= 512)
NBANK = NSLOT // SPB  # 5
NQR = 96              # q rows (90 real + 6 structural pads via zero q)

# a-groups: row offset a = (ky-1)+(u-1); (a, qbase, t0=T-slot start, n)
AGROUPS = [(-2, 0, 0, 13), (-1, 13, 0, 20), (0, 33, 0, 30),
           (1, 63, 10, 20), (2, 83, 17, 13)]

# T-build runs: (shift, jlo, jhi) same shift, within one PSUM bank
T_RUNS = []
for _ky in range(3):
    _base = _ky * NSPK
    for (_s, _lo, _hi) in ((-2, 0, 1), (-1, 1, 3), (0, 3, 6), (1, 6, 8),
                           (2, 8, 10)):
        _j = _base + _lo
        _e = _base + _hi
        while _j < _e:
            _be = (_j // SPB + 1) * SPB
            T_RUNS.append((_s, _j, min(_e, _be)))
            _j = min(_e, _be)

COL_TILES = [(PADC, 128), (PADC + 128, 128), (PADC + 256, 64)]


def _psum_col(slot):
    return (slot // SPB) * 512 + (slot % SPB) * O


def build_module():
    nc = bacc.Bacc("TRN2", target_bir_lowering=False, debug=False,
                   num_devices=N_CORES)

    img_d = nc.dram_tensor("img", [C, NPIX_I], F16, kind="ExternalInput")
    feat_d = nc.dram_tensor("feat", [C, NPIX_F], F16, kind="ExternalInput")
    wts_d = nc.dram_tensor("wts", [C, NSLOT * O], F16, kind="ExternalInput")
    offw_d = nc.dram_tensor("offw", [C, 9 * OC], F16, kind="ExternalInput")
    offb_d = nc.dram_tensor("offb", [OC, 1], F32, kind="ExternalInput")
    repy_d = nc.dram_tensor("repy", [OC, NQR], F16, kind="ExternalInput")
    repx_d = nc.dram_tensor("repx", [OC, NQR], F16, kind="ExternalInput")
    biasu_d = nc.dram_tensor("biasu", [NQR, 1], F32, kind="ExternalInput")
    biasv_d = nc.dram_tensor("biasv", [NQR, 1], F32, kind="ExternalInput")
    dmat_d = nc.dram_tensor("dmat", [NQR, 2 * NQR], F16, kind="ExternalInput")
    out_d = nc.dram_tensor("out", [RS * W, O], F32, kind="ExternalOutput")
    offs_d = nc.dram_tensor("offs", [OC, RS * W], F16, kind="ExternalOutput")

    NUNIT = RS * 3

    with tile.TileContext(nc) as tc, ExitStack() as ctx:
        const = ctx.enter_context(tc.tile_pool(name="const", bufs=1))
        big = ctx.enter_context(tc.tile_pool(name="big", bufs=1))

        wts = const.tile([C, NSLOT * O], F16)
        nc.sync.dma_start(wts[:], wts_d[:])
        offw = const.tile([C, 9 * OC], F16)
        nc.sync.dma_start(offw[:], offw_d[:])
        offb = const.tile([OC, 1], F32)
        nc.sync.dma_start(offb[:], offb_d[:])
        repy = const.tile([OC, NQR], F16)
        nc.sync.dma_start(repy[:], repy_d[:])
        repx = const.tile([OC, NQR], F16)
        nc.sync.dma_start(repx[:], repx_d[:])
        biasu = const.tile([NQR, 1], F32)
        nc.sync.dma_start(biasu[:], biasu_d[:])
        biasv = const.tile([NQR, 1], F32)
        nc.sync.dma_start(biasv[:], biasv_d[:])
        dmat = const.tile([NQR, 2 * NQR], F16)
        nc.sync.dma_start(dmat[:], dmat_d[:])

        imgh = big.tile([C, NPIX_I], F16)
        nc.sync.dma_start(imgh[:], img_d[:])
        q2 = big.tile([128, NUNIT * 2 * NQR], F16)

        # ---------------- phase 1: offsets -> per-pixel q2 rows ----------
        with tc.tile_pool(name="featp", bufs=1) as featp, \
             tc.tile_pool(name="ps_off", bufs=2, space="PSUM") as ps_off, \
             tc.tile_pool(name="ps_rep", bufs=2, space="PSUM") as ps_rep, \
             tc.tile_pool(name="ps_tr", bufs=2, space="PSUM") as ps_tr, \
             tc.tile_pool(name="sc", bufs=3) as sc:
            feat = featp.tile([C, NPIX_F], F16)
            nc.sync.dma_start(feat[:], feat_d[:])

            for r in range(RS):
                fbase = (r + 1) * WP + PADC
                po = ps_off.tile([OC, W], F32, tag="po")
                for t in range(9):
                    d = (t // 3 - 1) * WP + (t % 3 - 1)
                    nc.tensor.matmul(
                        po[:, :],
                        offw[:, t * OC:(t + 1) * OC],
                        feat[:, fbase + d: fbase + d + W],
                        start=(t == 0), stop=(t == 8))
                offs = sc.tile([OC, W], F16, tag="offs")
                nc.vector.tensor_scalar(
                    out=offs[:], in0=po[:, :], scalar1=offb[:], scalar2=None,
                    op0=mybir.AluOpType.add)
                nc.sync.dma_start(offs_d[:, r * W:(r + 1) * W], offs[:])

                ta = {}
                for (rep, bia, nm) in ((repy, biasu, "ty"), (repx, biasv,
                                                            "tx")):
                    pr = ps_rep.tile([NQR, W], F32, tag="pr")
                    nc.tensor.matmul(pr[:, :], rep[:, :], offs[:],
                                     start=True, stop=True)
                    tt = sc.tile([NQR, W], F16, tag=nm)
                    nc.scalar.activation(
                        tt[:, :], pr[:, :],
                        mybir.ActivationFunctionType.Abs,
                        bias=bia[:], scale=1.0)
                    nc.scalar.activation(
                        tt[:, :], tt[:, :],
                        mybir.ActivationFunctionType.Relu,
                        bias=1.0, scale=-1.0)
                    ta[nm] = tt
                qrow = sc.tile([NQR, W], F16, tag="qrow")
                nc.vector.tensor_tensor(out=qrow[:], in0=ta["ty"][:],
                                        in1=ta["tx"][:],
                                        op=mybir.AluOpType.mult)

                for ct, (c0, tw) in enumerate(COL_TILES):
                    unit = r * 3 + ct
                    ptq = ps_tr.tile([128, 2 * NQR], F32, tag="ptq")
                    nc.tensor.matmul(
                        ptq[:tw, :], qrow[:, c0 - PADC: c0 - PADC + tw],
                        dmat[:, :], start=True, stop=True)
                    nc.scalar.copy(
                        q2[:tw, unit * 2 * NQR: (unit + 1) * 2 * NQR],
                        ptq[:tw, :])

        # ---------------- phase 2: T slabs + combine ---------------------
        with tc.tile_pool(name="ps_T", bufs=1, space="PSUM") as ps_T, \
             tc.tile_pool(name="tpool", bufs=8) as tpool, \
             tc.tile_pool(name="apool", bufs=2) as apool, \
             tc.tile_pool(name="rpool", bufs=2) as rpool:

            for ct, (c0, tw) in enumerate(COL_TILES):
                t_tiles = {}

                def build_T(rp, c0=c0, tw=tw, t_tiles=t_tiles):
                    base = (rp + HALO) * WP + c0
                    pT = ps_T.tile([128, NBANK * 512], F32, tag="pT")
                    for (s, jlo, jhi) in T_RUNS:
                        nc.tensor.matmul(
                            pT[:tw, _psum_col(jlo):
                                    _psum_col(jlo) + (jhi - jlo) * O],
                            imgh[:, base + s: base + s + tw],
                            wts[:, jlo * O: jhi * O],
                            start=True, stop=True)
                    tsb = tpool.tile([128, NSLOT * O], F16, tag="tsb")
                    for bk in range(NBANK):
                        nc.scalar.copy(
                            tsb[:tw, bk * SPB * O: (bk + 1) * SPB * O],
                            pT[:tw, bk * 512: bk * 512 + SPB * O])
                    t_tiles[rp] = tsb

                for rp in range(-2, 2):
                    build_T(rp)
                for r in range(RS):
                    build_T(r + 2)
                    unit = r * 3 + ct
                    qoff = unit * 2 * NQR
                    qa = q2[:tw, :]

                    prod = apool.tile([128, NQR * O], F16, tag="prod")
                    pa = prod[:tw, :]
                    for (a, qb, t0, n) in AGROUPS:
                        ts = t_tiles[r + a][:tw, :]
                        in0 = AP(ts.tensor, ts.offset + t0 * O,
                                 [ts.ap[0], [O, n], [2, O // 2], [1, 2]])
                        in1 = AP(qa.tensor, qa.offset + qoff + 2 * qb,
                                 [qa.ap[0], [2, n], [0, O // 2], [1, 2]])
                        outp = AP(pa.tensor, pa.offset + qb * O,
                                  [pa.ap[0], [O, n], [2, O // 2], [1, 2]])
                        nc.vector.tensor_tensor(
                            out=outp, in0=in0, in1=in1,
                            op=mybir.AluOpType.mult)

                    # fp16 fold tree 96 -> 3, then 2 finishing adds
                    def fold(src, n_el, dst):
                        half = n_el // 2
                        nc.vector.tensor_tensor(
                            out=dst[:tw, 0:half],
                            in0=src[:tw, 0:half],
                            in1=src[:tw, half:n_el],
                            op=mybir.AluOpType.add)

                    h48 = rpool.tile([128, 48 * O], F16, tag="h48")
                    fold(prod, 96 * O, h48)
                    h24 = rpool.tile([128, 24 * O], F16, tag="h24")
                    fold(h48, 48 * O, h24)
                    h12 = rpool.tile([128, 12 * O], F16, tag="h12")
                    fold(h24, 24 * O, h12)
                    h6 = rpool.tile([128, 6 * O], F16, tag="h6")
                    fold(h12, 12 * O, h6)
                    h3 = rpool.tile([128, 3 * O], F16, tag="h3")
                    fold(h6, 6 * O, h3)
                    h1 = rpool.tile([128, O], F16, tag="h1")
                    fold(h3, 2 * O, h1)
                    acc = rpool.tile([128, O], F32, tag="acc")
                    nc.vector.tensor_tensor(
                        out=acc[:tw, :], in0=h1[:tw, :],
                        in1=h3[:tw, 2 * O:3 * O], op=mybir.AluOpType.add)

                    orow = r * W + (c0 - PADC)
                    nc.sync.dma_start(out_d[orow:orow + tw, :], acc[:tw, :])

    nc.compile()
    return nc


# ------------------------- host side -------------------------

_nc_cache = [None]


def _get_nc():
    if _nc_cache[0] is None:
        _nc_cache[0] = build_module()
    return _nc_cache[0]


def _consts(weight, off_w, off_b):
    # wk[k, c, o]: block-diag group conv weights for tap k
    wk = np.zeros((NK, C, O), np.float32)
    for g in range(9):
        for og in range(8):
            for cg in range(8):
                for k in range(NK):
                    wk[k, g * 8 + cg, g * 8 + og] = weight[
                        g * 8 + og, cg, k // 3, k % 3]
    # wts columns: slot-major [slot(30) x O]; slot = ky*10 + i (SLOT9 order)
    wts = np.zeros((C, NSLOT * O), np.float16)
    for ky in range(3):
        for i, (s, kx, v) in enumerate(SLOT9):
            k = ky * 3 + kx
            j = ky * NSPK + i
            wts[:, j * O:(j + 1) * O] = wk[k].astype(np.float16)

    offw = np.zeros((C, 9 * OC), np.float16)
    for t in range(9):
        offw[:, t * OC:(t + 1) * OC] = off_w[:, :, t // 3, t % 3].T

    # q rows: for each a-group, row qbase+j maps to T-slot t0+j
    repy = np.zeros((OC, NQR), np.float16)
    repx = np.zeros((OC, NQR), np.float16)
    biasu = np.full((NQR, 1), -3.0, np.float32)
    biasv = np.full((NQR, 1), -3.0, np.float32)
    for (a, qb, t0, n) in AGROUPS:
        for j in range(n):
            slot = t0 + j
            ky, i = slot // NSPK, slot % NSPK
            if i > 8:
                continue
            u = a - (ky - 1) + 1
            if u < 0 or u > 2:
                continue
            s, kx, v = SLOT9[i]
            k = ky * 3 + kx
            row = qb + j
            repy[2 * k, row] = 1.0
            repx[2 * k + 1, row] = 1.0
            biasu[row] = -(u - 1)
            biasv[row] = -(v - 1)
    dmat = np.zeros((NQR, 2 * NQR), np.float16)
    for j in range(NQR):
        dmat[j, 2 * j] = 1.0
        dmat[j, 2 * j + 1] = 1.0
    return {
        "wts": wts, "offw": offw,
        "offb": off_b.reshape(OC, 1).astype(np.float32),
        "repy": repy, "repx": repx, "biasu": biasu, "biasv": biasv,
        "dmat": dmat,
    }


def _slab(x_b, halo, rows):
    out = []
    for q in range(NQ):
        s = np.zeros((C, rows, WP), np.float16)
        lo, hi = q * RS - halo, q * RS + RS + halo
        clo, chi = max(lo, 0), min(hi, H)
        s[:, clo - lo: clo - lo + (chi - clo), PADC:PADC + W] = x_b[:, clo:chi]
        out.append(np.ascontiguousarray(s.reshape(C, rows * WP)))
    return out


def _patch_outliers(out, inp, weight, offs):
    """Fix (tap,pixel) events where |offset| >= 1: the 3x3 tent window on the
    device truncates them. offs: [B, OC, H, W] float32 (device fp16 values)."""
    dy = offs[:, 0::2]  # [B, 9, H, W]
    dx = offs[:, 1::2]
    TH = 0.9995
    bs, ks, ys, xs = np.nonzero((np.abs(dy) > TH) | (np.abs(dx) > TH))
    for b, k, y, x in zip(bs, ks, ys, xs):
        ky, kx = k // 3, k % 3
        dyv = float(dy[b, k, y, x])
        dxv = float(dx[b, k, y, x])
        py = y + ky - 1 + dyv
        px = x + kx - 1 + dxv
        # exact bilinear with zero outside
        y0, x0 = int(np.floor(py)), int(np.floor(px))
        wy1, wx1 = py - y0, px - x0
        s_true = np.zeros(C, np.float32)
        for yi, wy in ((y0, 1.0 - wy1), (y0 + 1, wy1)):
            for xi, wx in ((x0, 1.0 - wx1), (x0 + 1, wx1)):
                if 0 <= yi < H and 0 <= xi < W and wy * wx != 0.0:
                    s_true += (wy * wx) * inp[b, :, yi, xi]
        # what the device's truncated 3x3 tent computed
        s_kern = np.zeros(C, np.float32)
        for u in (-1, 0, 1):
            tyv = max(0.0, 1.0 - abs(dyv - u))
            if tyv == 0.0:
                continue
            for v in (-1, 0, 1):
                txv = max(0.0, 1.0 - abs(dxv - v))
                if txv == 0.0:
                    continue
                yy, xx = y + ky - 1 + u, x + kx - 1 + v
                if 0 <= yy < H and 0 <= xx < W:
                    s_kern += (tyv * txv) * inp[b, :, yy, xx]
        delta = (s_true - s_kern).reshape(9, 8)
        g = np.arange(O) // 8
        corr = (weight[:, :, ky, kx] * delta[g]).sum(axis=1)
        out[b, :, y, x] += corr


def kernel(input, offset_feat, weight, off_w, off_b):
    input = np.asarray(input, np.float32)
    offset_feat = np.asarray(offset_feat, np.float32)
    weight = np.asarray(weight, np.float32)
    off_w = np.asarray(off_w, np.float32)
    off_b = np.asarray(off_b, np.float32)

    nc = _get_nc()
    consts = _consts(weight, off_w, off_b)
    in_maps = []
    for b in range(B):
        imgs = _slab(input[b], HALO, RSP)
        feats = _slab(offset_feat[b], 1, FROWS)
        for q in range(NQ):
            m = dict(consts)
            m["img"] = imgs[q]
            m["feat"] = feats[q]
            in_maps.append(m)

    res = bass_utils.run_bass_kernel_spmd(
        nc, in_maps, core_ids=list(range(N_CORES)))

    out = np.empty((B, O, H, W), np.float32)
    offs = np.empty((B, OC, H, W), np.float32)
    for ci in range(N_CORES):
        b, q = ci // NQ, ci % NQ
        o = res.results[ci]["out"]
        out[b, :, q * RS:(q + 1) * RS, :] = (
            o.reshape(RS, W, O).transpose(2, 0, 1))
        offs[b, :, q * RS:(q + 1) * RS, :] = (
            res.results[ci]["offs"].reshape(OC, RS, W).astype(np.float32))

    _patch_outliers(out, input, weight, offs)
    return out


if __name__ == "__main__":
    import reference as ref
    inputs = {k: np.asarray(v) for k, v in ref.setup_inputs().items()}
    got = kernel(**inputs)
    print("out", got.shape, got.dtype)
